# revision 2
# baseline (speedup 1.0000x reference)
"""Multi-head self-attention (CMHSAttn) Trainium2 kernel, ACT+DVE split exp.

Sharding: one head per NeuronCore (8 cores), data parallel, no collectives.
Per-core pipeline (HW-measured best 129.6us vs 151.5us all-ACT baseline):
  - qkv projection on PE with PSUM->SBUF copies alternating ACT/DVE; kt
    projected right after qt chunk-group 0 (scores need ALL kt columns but
    only the current qt chunk); x DMA'd in quarters so projection starts
    after the first quarter lands.
  - Scores processed in slots of 2 k-tiles (FD=1024): 12 slots/chunk on ACT
    (exact exp, 1.23us/op measured) + 4 slots/chunk on DVE (Schraudolph exp:
    one tensor_scalar x*C1+C2 -> int16, bitcast bf16; 1.2us/op measured,
    ~3.3% pointwise sawtooth, ~6e-3 end-to-end).
  - Dedicated PSUM pools (ACT 2x2 banks, DVE 1x2, O'' accum 2x1) so the DVE
    path never blocks the ACT score rotation.
  - Static emission schedule with per-engine OV lag (A=2, D=4): the OV
    matmuls of a slot are emitted several slots later so the in-order PE
    never head-of-line blocks on a pending exp (LAG_A=2 measured 129.6us vs
    245us at LAG_A=1 with the same other knobs - the schedule sits near
    stall cliffs; these knobs are HW-tuned, treat as load-bearing).
  - Normalize per q-chunk on DVE (iterative reciprocal 3.4us + mul 0.8us),
    emitted late (slot 8 of the next chunk) to fit the DVE idle window
    between Schraudolph ops. exp(-ln d) on ACT measured slower: walrus
    re-inserts activation-table loads per Ln/Exp switch (~2.7us each).
  - Custom-DVE ops (reciprocal_approx_fast) unavailable: this walrus build
    fails with "ISA wrong length" on InstCustomDveAnt encodings.
"""

import math

import ml_dtypes
import numpy as np

D_MODEL = 128
N = 4096
DH = 16
NH = 8
QC = 512
NQC = N // QC  # 8
KT = 128
NKJ = N // KT  # 32
SCALE = 1.0 / math.sqrt(D_MODEL)

# Schraudolph bf16-int16 exp constants (truncating fp32->int16 convert)
C1 = SCALE * (128.0 / math.log(2.0))
C2 = 127.0 * 128.0 - 5.125

# ---- schedule knobs ----
ACT_W = 2  # k-tiles per ACT slot (1..3); PSUM bufs are sized to this
PSA_BUFS = 2  # ACT score-psum rotation depth
D_W = 2  # k-tiles per DVE slot (1 or 2)
PSD_BUFS = 1  # DVE score-psum rotation depth
D_COUNT = 4  # DVE slots per chunk
OV_LAG_A = 2  # slots between an ACT slot's emission and its OV emission
OV_LAG_D = 4  # same for DVE slots (Schraudolph op is ~2.5x slower)
NORM_AFTER_SLOT = 8  # norm(c) emitted after this slot index of chunk c+1
# normalize mode: "lnexp" = ACT ln + ACT exp(-x) + DVE mul (the installed
# walrus cannot encode custom-DVE ops, so reciprocal_approx_fast is out);
# "recip" = DVE iterative reciprocal + DVE mul (baseline-proven, costly)
NORM_MODE = "recip"


def _chunk_slots():
    """Per-chunk slot list: [(eng, [kj, ...]), ...] covering kj 0..31.
    D slots are D_W wide; ACT slots are ACT_W wide (plus one remainder slot).
    D slots are spread evenly among the ACT slots."""
    d_tiles = D_W * D_COUNT
    a_tiles = NKJ - d_tiles
    a_widths = []
    rem = a_tiles
    while rem > 0:
        w = min(ACT_W, rem)
        a_widths.append(w)
        rem -= w
    # smallest ACT slot first (ramps the scalar engine with least latency)
    a_widths.sort()
    seq = [("A", w) for w in a_widths]
    for i in reversed(range(D_COUNT)):
        idx = 1 + (i * len(a_widths)) // max(D_COUNT, 1)
        seq.insert(min(idx, len(seq)), ("D", D_W))
    slots = []
    kj = 0
    for eng, w in seq:
        slots.append((eng, list(range(kj, kj + w))))
        kj += w
    assert kj == NKJ, (kj, slots)
    return slots

_NC_CACHE = {}


def _validate_schedule(ev, slots):
    seen_slot = set()
    seen_ov = {}
    seen_norm = set()
    for e in ev:
        if e[0] == "slot":
            seen_slot.add((e[1], e[2]))
        elif e[0] == "ov":
            assert (e[1], e[2]) in seen_slot, f"ov before slot: {e}"
            assert e[1] not in seen_norm, f"ov after norm: {e}"
            seen_ov.setdefault(e[1], []).extend(e[3])
        else:
            assert sorted(seen_ov.get(e[1], [])) == list(range(NKJ)), (
                f"norm before full OV coverage: chunk {e[1]}"
            )
            seen_norm.add(e[1])
    assert len(seen_norm) == NQC
    assert len(seen_slot) == NQC * len(slots)


def _build_schedule():
    """Flat list of emission events. Event shapes:
    ("slot", qc, si, eng, kjs), ("ov", qc, si, kjs, start, stop), ("norm", qc)."""
    slots = _chunk_slots()
    ev = []
    pend = []  # (due_flat, qc, si, kjs)
    flat = 0
    released = [0] * NQC  # OV slots released per chunk
    normed = [False] * NQC
    for qc in range(NQC):
        for si, (eng, kjs) in enumerate(slots):
            ev.append(("slot", qc, si, eng, kjs))
            flat += 1
            lag = OV_LAG_D if eng == "D" else OV_LAG_A
            # due = flat + lag: lag=1 means the OV is released after the
            # NEXT slot's score MMs (the cross-boundary carry that keeps the
            # in-order PE from head-of-line blocking on the pending exp)
            pend.append((flat + lag, qc, si, kjs))
            # release due OVs (FIFO within same due)
            ready = [p for p in pend if p[0] <= flat]
            for p in ready:
                pend.remove(p)
                ev.append(("ov", p[1], p[2], p[3], False, False))
                released[p[1]] += 1
            # norm(c-1) once all its OVs are out and we're past the gate slot
            if (
                qc > 0
                and si >= NORM_AFTER_SLOT
                and not normed[qc - 1]
                and released[qc - 1] == len(slots)
            ):
                ev.append(("norm", qc - 1))
                normed[qc - 1] = True
    pend.sort(key=lambda p: p[0])
    for p in pend:
        ev.append(("ov", p[1], p[2], p[3], False, False))
        released[p[1]] += 1
    for qc in range(NQC):
        if not normed[qc]:
            assert released[qc] == len(slots)
            ev.append(("norm", qc))
            normed[qc] = True
    _validate_schedule(ev, slots)
    # assign start/stop per chunk by emission order of OVs
    first = {}
    last = {}
    for i, e in enumerate(ev):
        if e[0] == "ov":
            first.setdefault(e[1], i)
            last[e[1]] = i
    out = []
    for i, e in enumerate(ev):
        if e[0] == "ov":
            out.append((e[0], e[1], e[2], e[3], i == first[e[1]], i == last[e[1]]))
        else:
            out.append(e)
    return out


def _build_nc(legalize=True, loop_reps=None):
    import concourse.bass as bass
    import concourse.mybir as mybir
    from concourse.tile import TileContext

    fp32 = mybir.dt.float32
    bf16 = mybir.dt.bfloat16
    i16 = mybir.dt.int16
    EXP = mybir.ActivationFunctionType.Exp
    COPY = mybir.ActivationFunctionType.Copy
    LN = mybir.ActivationFunctionType.Ln

    nc = bass.Bass(name="cmhs_attn_head")
    xb = nc.dram_tensor("xb", [D_MODEL, N], bf16, kind="ExternalInput")
    wq = nc.dram_tensor("wq", [D_MODEL, 128], bf16, kind="ExternalInput")
    wk = nc.dram_tensor("wk", [D_MODEL, 128], bf16, kind="ExternalInput")
    wv = nc.dram_tensor("wv", [D_MODEL, DH], bf16, kind="ExternalInput")
    out = nc.dram_tensor("out", [DH, N], fp32, kind="ExternalOutput")

    sched = _build_schedule()
    banks = PSA_BUFS * ACT_W + (PSD_BUFS * D_W if D_COUNT else 0) + 2
    assert banks <= 8, f"PSUM over budget: {banks}"

    with (
        TileContext(nc) as tc,
        tc.tile_pool(name="const", bufs=1) as cpool,
        tc.tile_pool(name="pa", bufs=4) as papb,
        tc.tile_pool(name="pd", bufs=4) as pdpb,
        tc.tile_pool(name="small", bufs=3) as mpool,
        tc.tile_pool(name="psa", bufs=PSA_BUFS, space="PSUM") as papool,
        tc.tile_pool(name="psd", bufs=PSD_BUFS, space="PSUM") as pdpool,
        tc.tile_pool(name="po", bufs=2, space="PSUM") as popool,
    ):
        xb_sb = cpool.tile([D_MODEL, N], bf16, name="xb_sb")
        wq_sb = cpool.tile([D_MODEL, 128], bf16, name="wq_sb")
        wk_sb = cpool.tile([D_MODEL, 128], bf16, name="wk_sb")
        wv_sb = cpool.tile([D_MODEL, DH], bf16, name="wv_sb")
        qt = cpool.tile([D_MODEL, N], bf16, name="qt")
        kt = cpool.tile([D_MODEL, N], bf16, name="kt")
        v2 = cpool.tile([D_MODEL, NKJ * 48], bf16, name="v2")
        v2_v = v2.rearrange("p (j t) -> p j t", t=48)

        def proj_qk_group(dst, w_sb, c0, cn, eng):
            # project x-chunks c0..c0+cn of q^T or k^T (replicated rows)
            pj = papool.tile([D_MODEL, ACT_W * QC], fp32, name="pj", tag="a")
            for t in range(cn):
                c = c0 + t
                nc.tensor.matmul(
                    pj[:, t * QC : (t + 1) * QC],
                    lhsT=w_sb[:],
                    rhs=xb_sb[:, c * QC : (c + 1) * QC],
                    start=True,
                    stop=True,
                )
            if eng == "A":
                nc.scalar.activation(
                    dst[:, c0 * QC : (c0 + cn) * QC],
                    pj[:, : cn * QC],
                    COPY,
                    scale=1.0,
                )
            else:
                nc.vector.tensor_copy(
                    out=dst[:, c0 * QC : (c0 + cn) * QC], in_=pj[:, : cn * QC]
                )

        def proj_v():
            if D_COUNT and D_W * QC >= NKJ * DH:
                vp = pdpool.tile([D_MODEL, D_W * QC], fp32, name="vp", tag="d")
            else:
                vp = papool.tile([D_MODEL, ACT_W * QC], fp32, name="vp", tag="a")
            vp_v = vp[:, : NKJ * DH].rearrange("p (j t) -> p j t", t=DH)
            for kj in range(NKJ):
                nc.tensor.matmul(
                    vp[:, kj * DH : (kj + 1) * DH],
                    lhsT=xb_sb[:, kj * KT : (kj + 1) * KT],
                    rhs=wv_sb[:],
                    start=True,
                    stop=True,
                )
            nc.vector.tensor_copy(
                out=v2_v[:, 0:NKJ, 0:DH], in_=vp_v[:, 0:NKJ, :]
            )

        def body():
            nc.vector.memset(v2_v[:, :, DH:32], 0.0)
            nc.vector.memset(v2_v[:, :, 32:48], 1.0)

            nc.sync.dma_start(out=wq_sb[:], in_=wq[:])
            nc.sync.dma_start(out=wk_sb[:], in_=wk[:])
            nc.sync.dma_start(out=wv_sb[:], in_=wv[:])
            for q in range(4):
                nc.sync.dma_start(
                    out=xb_sb[:, q * (N // 4) : (q + 1) * (N // 4)],
                    in_=xb[:, q * (N // 4) : (q + 1) * (N // 4)],
                )

            # warm the ACT exp table at t=0
            warm = mpool.tile([1, 32], bf16, name="warm", tag="warm")
            nc.vector.memset(warm[:], 0.25)
            nc.scalar.activation(warm[:], warm[:], EXP, scale=SCALE)

            # projection groups of ACT_W chunks; copies alternate ACT/DVE
            pg = []
            c = 0
            while c < NQC:
                w = min(ACT_W, NQC - c)
                pg.append((c, w))
                c += w
            # qt chunk-group 0 first, then all of kt (scores for q-chunk 0
            # need qt chunk 0 but ALL kt columns), then the rest of qt
            proj_qk_group(qt, wq_sb, pg[0][0], pg[0][1], "A")
            for i, (c0, cn) in enumerate(pg):
                proj_qk_group(kt, wk_sb, c0, cn, "A" if i % 2 == 1 else "D")
            for i, (c0, cn) in enumerate(pg[1:]):
                proj_qk_group(qt, wq_sb, c0, cn, "A" if i % 2 == 0 else "D")
            proj_v()

            slot_ps = {}  # (qc, si) -> score psum tile
            slot_pb = {}  # (qc, si) -> bf16 view of P tile
            chunk_o2 = {}

            def get_o2(qc):
                if qc not in chunk_o2:
                    chunk_o2[qc] = popool.tile([48, QC], fp32, name="o2", tag="o")
                return chunk_o2[qc]

            for e in sched:
                if e[0] == "slot":
                    _, qc, si, eng, kjs = e
                    qs = qc * QC
                    w = len(kjs)
                    if eng == "A":
                        sps = papool.tile(
                            [D_MODEL, ACT_W * QC], fp32, name="sps", tag="a"
                        )
                    else:
                        sps = pdpool.tile(
                            [D_MODEL, D_W * QC], fp32, name="sps", tag="d"
                        )
                    for t, kj in enumerate(kjs):
                        ro = 32 * t
                        nc.tensor.matmul(
                            sps[:, t * QC : (t + 1) * QC],
                            lhsT=kt[ro : ro + DH, kj * KT : (kj + 1) * KT],
                            rhs=qt[ro : ro + DH, qs : qs + QC],
                            start=True,
                            stop=True,
                        )
                    if eng == "A":
                        pb = papb.tile(
                            [D_MODEL, ACT_W * QC], bf16, name="pb", tag="p"
                        )
                        nc.scalar.activation(
                            pb[:, : w * QC], sps[:, : w * QC], EXP, scale=SCALE
                        )
                        slot_pb[(qc, si)] = pb
                    else:
                        pbd = pdpb.tile(
                            [D_MODEL, D_W * QC], i16, name="pbd", tag="pd"
                        )
                        nc.vector.tensor_scalar(
                            out=pbd[:, : w * QC],
                            in0=sps[:, : w * QC],
                            scalar1=float(C1),
                            scalar2=float(C2),
                            op0=mybir.AluOpType.mult,
                            op1=mybir.AluOpType.add,
                        )
                        slot_pb[(qc, si)] = pbd.bitcast(bf16)
                    slot_ps[(qc, si)] = sps
                elif e[0] == "ov":
                    _, qc, si, kjs, first, lastf = e
                    o2 = get_o2(qc)
                    pb = slot_pb.pop((qc, si))
                    for t, kj in enumerate(kjs):
                        nc.tensor.matmul(
                            o2[:],
                            lhsT=v2[:, kj * 48 : kj * 48 + 48],
                            rhs=pb[:, t * QC : (t + 1) * QC],
                            start=(first and t == 0),
                            stop=(lastf and t == len(kjs) - 1),
                            skip_group_check=True,
                        )
                else:  # norm
                    _, qc = e
                    o2 = chunk_o2.pop(qc)
                    rcp = mpool.tile([DH, QC], fp32, name="rcp", tag="rcp")
                    if NORM_MODE == "lnexp":
                        # 1/d = exp(-ln d); Ln and Exp share the
                        # natural_log_exp_and_others ACT table set
                        lnd = mpool.tile([DH, QC], fp32, name="lnd", tag="lnd")
                        nc.scalar.activation(
                            lnd[:], o2[32:48, :], LN, scale=1.0
                        )
                        nc.scalar.activation(rcp[:], lnd[:], EXP, scale=-1.0)
                    else:
                        nc.vector.reciprocal(rcp[:], o2[32:48, :])
                    ob = mpool.tile([DH, QC], fp32, name="ob", tag="ob")
                    nc.vector.tensor_mul(ob[:], o2[0:DH, :], rcp[:])
                    nc.sync.dma_start(
                        out=out[:, qc * QC : (qc + 1) * QC], in_=ob[:]
                    )

        if loop_reps is None:
            body()
        else:
            with tc.For_i(0, loop_reps, 1):
                body()

    if legalize:
        _legalize_pe_waits(nc)
    return nc


def _legalize_pe_waits(nc):
    """Hoist extra sync-waits (HW instruction formats have one wait slot)
    onto EventSemaphore instructions on the same engine queue."""
    import concourse.mybir as mybir

    skip = {"EventSemaphore", "Call"}
    n = 0
    for blk in nc.m.functions[0].blocks:
        insts = blk.instructions
        out = []
        changed = False
        for inst in insts:
            si = getattr(inst, "sync_info", None)
            if (
                inst.opcode not in skip
                and si is not None
                and si.on_wait
                and len(si.on_wait) > 1
            ):
                waits = list(si.on_wait)
                for w in waits[:-1]:
                    ev = mybir.InstEventSemaphore(
                        name=f"hoistwait_{inst.name}_{n}", ins=[], outs=[]
                    )
                    n += 1
                    ev.engine = inst.engine
                    ev.sync_info = mybir.SyncInfo(on_wait=[w], on_update=[])
                    out.append(ev)
                si.on_wait = [waits[-1]]
                changed = True
            out.append(inst)
        if changed:
            blk.instructions = out
    return nc


def _get_nc():
    if "nc" not in _NC_CACHE:
        _NC_CACHE["nc"] = _build_nc()
    return _NC_CACHE["nc"]


def make_in_maps(x, W_qkv):
    """Host-side sharding: per-head input maps for the 8 cores."""
    bf16 = ml_dtypes.bfloat16
    x = np.asarray(x, dtype=np.float32).reshape(D_MODEL, N)
    W = np.asarray(W_qkv, dtype=np.float32)
    xb = np.ascontiguousarray(x.astype(bf16))
    in_maps = []
    for h in range(NH):
        Wq = W[48 * h : 48 * h + 16]
        Wk = W[48 * h + 16 : 48 * h + 32]
        Wv = W[48 * h + 32 : 48 * h + 48]
        wq_rep = np.zeros((D_MODEL, 128), dtype=bf16)
        wk_rep = np.zeros((D_MODEL, 128), dtype=bf16)
        for i in range(3):
            wq_rep[:, 32 * i : 32 * i + 16] = Wq.T.astype(bf16)
            wk_rep[:, 32 * i : 32 * i + 16] = Wk.T.astype(bf16)
        in_maps.append(
            {
                "xb": xb,
                "wq": wq_rep,
                "wk": wk_rep,
                "wv": np.ascontiguousarray(Wv.T.astype(bf16)),
            }
        )
    return in_maps


def run_spmd(x, W_qkv, **kwargs):
    from concourse.bass_utils import run_bass_kernel_spmd

    nc = _get_nc()
    in_maps = make_in_maps(x, W_qkv)
    return run_bass_kernel_spmd(nc, in_maps, core_ids=list(range(NH)), **kwargs)


def kernel(x, W_qkv):
    res = run_spmd(x, W_qkv)
    outs = [res.results[h]["out"] for h in range(NH)]
    full = np.concatenate(outs, axis=0)
    return np.ascontiguousarray(full.reshape(1, D_MODEL, 64, 64), dtype=np.float32)
